# revision 10
# baseline (speedup 1.0000x reference)
"""Masked multi-head self-attention on 8 Trainium2 NeuronCores.

Math (per batch element b, faithful to the reference up to fp rounding):
    q = blockdiag(Wq) @ Q ; k = blockdiag(Wk) @ K ; vT = Q-style grouped conv,
    logitsT[h][j, i] = sum_c k[h][c, j] * q[h][c, i]        (j = key pos, i = query pos)
    P~T[h][j, i]    = exp(logitsT + logmask[j])             (mask folded into exp bias;
                                                             softmax max-shift dropped --
                                                             logits are O(40), exp is safe)
    val[h][c, i]    = sum_j vT[h][j, c] * P~T[h][j, i]      (plus a ones column giving
                                                             n[i] = sum_j P~T[j, i])
    val_scaled      = val * (mask[i] / n[i])                 (per-head normalizer)
    outT[l, d]      = sum_j val_scaled[j, l] * WpT[j, d] + mask[l] * bp[d]

Sharding: pure data-parallel over batch (BS == 8 == n_cores), no collectives.
"""

import numpy as np

import concourse.bass as bass
import concourse.mybir as mybir
import concourse.tile as tile
from concourse.bass_utils import run_bass_kernel_spmd
from concourse.vector_clock import ScopedClock

# Problem shapes (hardcoded per contract).
BS, D, L, H = 8, 256, 1024, 8
DK = D // H            # 32
G = 2                  # channel groups of 128 (4 heads each)
JB = L // 128          # 8 key-position blocks
LB = L // 128          # 8 query-position blocks
NEG_BIG = -30000.0     # exp(x + NEG_BIG) == 0 for any realistic logit x
SHIFT = 20.0           # global exp shift: P~ and n scale by e^-SHIFT, P unchanged;
                       # guards fp32 overflow for logits up to ~108
F32 = mybir.dt.float32
BF16 = mybir.dt.bfloat16
VP = 34                # vones pitch: [v(32) | ones | pad], 4B-aligned in bf16

_CACHED = {}


def _patch_tile_drain():
    """walrus in this container rejects >1 sync wait on a TPB_CTRL Drain.
    Split the TileContext exit drain's waits across multiple drains."""
    if getattr(tile.TileContext, "_drain_patched", False):
        return

    def _drain_and_barrier(self, tick_clock, wait_clock):
        drain_inst = self.nc.sync.drain(fusable=False)
        wait_clock.add_sem_waits(
            drain_inst.ins, ScopedClock({None: tick_clock.global_clock})
        )
        si = drain_inst.ins.sync_info
        waits = list(si.on_wait or []) if si else []
        if len(waits) > 1:
            si.on_wait = waits[:1]
            drain_inst.ins.sync_info = si
            for w in waits[1:]:
                d2 = self.nc.sync.drain(fusable=False)
                d2.ins.sync_info = mybir.SyncInfo(on_wait=[w], on_update=[])
        self.nc.all_engine_barrier()
        assert self.sems is not None
        popped = self.nc._tile_sem_poison_stack.pop()
        assert popped is self._sem_poison
        self.nc.clear_and_free_semaphores(list(self.sems.allocated().values()))
        self.nc.all_engine_barrier()

    tile.TileContext._drain_and_barrier = _drain_and_barrier
    tile.TileContext._drain_patched = True


def _split_multi_waits(nc, cap=1):
    """This container's walrus accepts at most `cap` sync-wait commands per
    instruction. Hoist extra waits onto same-engine NoOps inserted directly
    before the instruction (engine queues are FIFO, so semantics are
    unchanged)."""
    k = 0
    for fn in nc.m.functions:
        for bb in fn.blocks:
            out = []
            for inst in bb.instructions:
                si = inst.sync_info
                waits = list(si.on_wait) if (si and si.on_wait) else []
                if len(waits) > cap:
                    for i in range(cap, len(waits), cap):
                        nop = mybir.InstNoOp(
                            name=f"waitnop-{k}", engine=inst.engine, ins=[],
                            outs=[],
                            sync_info=mybir.SyncInfo(
                                on_wait=waits[i:i + cap], on_update=[]),
                        )
                        k += 1
                        out.append(nop)
                    si.on_wait = waits[:cap]
                    inst.sync_info = si
                out.append(inst)
            bb.instructions = out


def _build_nc(repeat=1, skip=()):
    _patch_tile_drain()
    nc = bass.Bass()

    q_d = nc.declare_dram_parameter("q", [D, L], F32, isOutput=False)
    k_d = nc.declare_dram_parameter("k", [D, L], F32, isOutput=False)
    v_d = nc.declare_dram_parameter("v", [D, L], BF16, isOutput=False)
    mcols_d = nc.declare_dram_parameter("mcols", [128, JB], F32, isOutput=False)
    sel_d = nc.declare_dram_parameter("sel", [4, 128], BF16, isOutput=False)
    wq_d = nc.declare_dram_parameter("wq", [G, 128, 128], F32, isOutput=False)
    wk_d = nc.declare_dram_parameter("wk", [G, 128, 128], F32, isOutput=False)
    wv_d = nc.declare_dram_parameter("wv", [G, 128, 128], BF16, isOutput=False)
    wpt_d = nc.declare_dram_parameter("wpt", [G, 128, D], BF16, isOutput=False)
    bp_d = nc.declare_dram_parameter("bp", [1, D], BF16, isOutput=False)
    out_d = nc.declare_dram_parameter("out", [L, D], F32, isOutput=True)

    EXP = mybir.ActivationFunctionType.Exp
    COPY = mybir.ActivationFunctionType.Copy

    with tile.TileContext(nc) as tc:
        with tc.tile_pool(name="persist", bufs=1) as pp:
            # ---- persistent SBUF tiles -------------------------------------
            def ptile(tag, shape):
                return pp.tile(shape, F32, tag=tag, name=tag)

            qin = [ptile(f"qin{g}", [128, L]) for g in range(G)]
            kin = [ptile(f"kin{g}", [128, L]) for g in range(G)]

            wq_t = [ptile(f"wq{g}", [128, 128]) for g in range(G)]
            wk_t = [ptile(f"wk{g}", [128, 128]) for g in range(G)]
            wv_t = [pp.tile([128, 128], BF16, tag=f"wv{g}", name=f"wv{g}") for g in range(G)]
            wpt_t = [pp.tile([128, D], BF16, tag=f"wpt{g}", name=f"wpt{g}") for g in range(G)]
            bp_t = pp.tile([1, D], BF16, tag="bp", name="bp")
            ones_row = pp.tile([1, 128], BF16, tag="ones_row", name="ones_row")
            sel_t = pp.tile([4, 128], BF16, tag="sel", name="sel")
            mcols_t = ptile("mcols", [128, JB])
            logm_t = ptile("logm", [128, JB])
            qh = [ptile(f"qh{g}", [128, L]) for g in range(G)]       # conv'd q
            kh = [ptile(f"kh{g}", [128, L]) for g in range(G)]       # conv'd k
            # split-bf16 halves of qh/kh: x = x1 + x2 with x1 = bf16(x);
            # logits = k1*q1 + k1*q2 + k2*q1 (+k2*q2 dropped, ~2^-16 rel)
            q1a = [pp.tile([128, L], BF16, tag=f"q1a{g}", name=f"q1a{g}")
                   for g in range(G)]
            q2a = [pp.tile([128, L], BF16, tag=f"q2a{g}", name=f"q2a{g}")
                   for g in range(G)]
            k1a = [pp.tile([128, L], BF16, tag=f"k1a{g}", name=f"k1a{g}")
                   for g in range(G)]
            k2a = [pp.tile([128, L], BF16, tag=f"k2a{g}", name=f"k2a{g}")
                   for g in range(G)]
            scr = ptile("scr", [128, L])                             # residual scratch
            vinb = [pp.tile([128, L], BF16, tag=f"vinb{g}", name=f"vinb{g}")
                    for g in range(G)]
            # [v_head | 1 | pad] stacks: per (group, jblk), bf16 for the PV matmul
            vones = [[pp.tile([128, 4 * VP], BF16, tag=f"vo{g}_{j}",
                              name=f"vo{g}_{j}") for j in range(JB)]
                     for g in range(G)]
            valk = [pp.tile([128, L], BF16, tag=f"valk{g}", name=f"valk{g}") for g in range(G)]   # raw val (bf16), K-tile layout
            valsc = [pp.tile([128, L], BF16, tag=f"valsc{g}", name=f"valsc{g}") for g in range(G)]  # normalized val, bf16 for proj
            nm = [ptile(f"nm{g}", [4, L]) for g in range(G)]         # per-head softmax sums
            nrow = [ptile(f"nrow{h}", [1, L]) for h in range(H)]     # aligned n bounce
            rm = [pp.tile([4, L], BF16, tag=f"rm{g}", name=f"rm{g}") for g in range(G)]  # 1/n rows (bf16)
            rsc = [ptile(f"rsc{g}", [4, L]) for g in range(G)]       # recip scratch

            for _rep in range(repeat):
                # ---- load everything -------------------------------------------
                for g in range(G):
                    nc.sync.dma_start(qin[g][:], q_d[128 * g:128 * (g + 1), :])
                    nc.sync.dma_start(wq_t[g][:], wq_d[g])
                    nc.sync.dma_start(kin[g][:], k_d[128 * g:128 * (g + 1), :])
                    nc.sync.dma_start(wk_t[g][:], wk_d[g])
                nc.sync.dma_start(mcols_t[:], mcols_d[:])
                for g in range(G):
                    nc.sync.dma_start(vinb[g][:], v_d[128 * g:128 * (g + 1), :])
                    nc.sync.dma_start(wv_t[g][:], wv_d[g])
                for g in range(G):
                    nc.sync.dma_start(wpt_t[g][:], wpt_d[g])
                nc.sync.dma_start(bp_t[:], bp_d[:])
                nc.vector.memset(ones_row[:], 1.0)
                nc.sync.dma_start(sel_t[:], sel_d[:])
                # logmask columns: (m - 1) * |NEG_BIG|  ->  0 or NEG_BIG
                nc.scalar.activation(logm_t[:], mcols_t[:], COPY,
                                     bias=NEG_BIG - SHIFT, scale=-NEG_BIG)

                # ---- phase A: grouped 1x1 convs ---------------------------------
                with tc.tile_pool(name=f"cpsum{_rep}", bufs=2, space="PSUM") as cps, \
                     tc.tile_pool(name=f"vtpsum{_rep}", bufs=2, space="PSUM") as vps:
                    for g in range(G):
                        qp = cps.tile([128, L], F32, tag="convp", name="convp")
                        for ih in range(2):
                            nc.tensor.matmul(qp[:, 512 * ih:512 * (ih + 1)], wq_t[g][:],
                                             qin[g][:, 512 * ih:512 * (ih + 1)])
                        nc.vector.tensor_copy(qh[g][:], qp[:])
                        kp = cps.tile([128, L], F32, tag="convp", name="convp")
                        for ih in range(2):
                            nc.tensor.matmul(kp[:, 512 * ih:512 * (ih + 1)], wk_t[g][:],
                                             kin[g][:, 512 * ih:512 * (ih + 1)])
                        nc.vector.tensor_copy(kh[g][:], kp[:])
                    # split qh/kh into bf16 high + bf16 residual
                    for g in range(G):
                        for full, hi_t, lo_t in ((qh[g], q1a[g], q2a[g]),
                                                 (kh[g], k1a[g], k2a[g])):
                            nc.vector.tensor_copy(hi_t[:], full[:])
                            nc.vector.tensor_sub(scr[:], full[:], hi_t[:])
                            nc.vector.tensor_copy(lo_t[:], scr[:])
                    # vT: per (g, lblk): (128 l x 128 heads*dk) = V_g[:, lblk].T @ blockdiag(WvT)
                    for g in range(G):
                        for j in range(JB):
                            vp = vps.tile([128, 128], F32, tag="vtp", name="vtp")
                            nc.tensor.matmul(vp[:], vinb[g][:, 128 * j:128 * (j + 1)],
                                             wv_t[g][:])
                            vo = vones[g][j]
                            vo3 = vo.rearrange("p (h c) -> p h c", c=VP)
                            nc.vector.memset(vo3[:, :, DK:DK + 1], 1.0)
                            vp3 = vp.rearrange("p (h c) -> p h c", c=DK)
                            nc.vector.tensor_copy(vo3[:, :, 0:DK], vp3[:])

                # ---- phase B: attention, one 4-head group at a time -------------
                # Superstep (g, j, ihalf): two PSUM tiles each holding two heads'
                # logitsT slices -> 4 QKT matmuls on distinct 32-row PE strips
                # (concurrent on HW) -> one exp per tile (FD=1024, bf16 out) ->
                # 4 bf16 PV matmuls (col-paired, M=33 incl. the n ones-column).
                with tc.tile_pool(name=f"qkt{_rep}", bufs=2, space="PSUM") as qkt_pool, \
                     tc.tile_pool(name=f"valp{_rep}", bufs=2, space="PSUM") as val_pool, \
                     tc.tile_pool(name=f"pt{_rep}", bufs=6) as pt_pool:
                    group_vals = []
                    pending_pv = None

                    def _drain_group(g_, vals_):
                        tail = g_ == G - 1
                        for pr in range(2):
                            for hi in range(2):
                                h = 4 * g_ + 2 * pr + hi
                                qoff = 64 * hi
                                co = 32 * (h % 4)
                                nc.vector.tensor_copy(valk[g_][co:co + 32, :],
                                                      vals_[pr][qoff:qoff + 32, :])
                                if tail:
                                    # ACT is idle once attention ends; keep the
                                    # critical tail chain off the busy DVE
                                    nc.scalar.activation(
                                        nrow[h][:],
                                        vals_[pr][qoff + 32:qoff + 33, :], COPY)
                                else:
                                    nc.vector.tensor_copy(
                                        nrow[h][:],
                                        vals_[pr][qoff + 32:qoff + 33, :])
                                nc.sync.dma_start(nm[g_][h % 4:h % 4 + 1, :],
                                                  nrow[h][:])
                        with nc.allow_low_precision(reason="softmax 1/n in bf16 is within the error gate"):
                            nc.vector.reciprocal(rm[g_][:], nm[g_][:])

                    def emit_pv(vals_, pts_, g_, j_, ih_):
                        for pr in range(2):
                            for hi in range(2):
                                hh = 2 * pr + hi
                                nc.tensor.matmul(
                                    vals_[pr][64 * hi:64 * hi + DK + 1,
                                              512 * ih_:512 * (ih_ + 1)],
                                    vones[g_][j_][:, VP * hh:VP * hh + DK + 1],
                                    pts_[pr][:, 512 * hi:512 * (hi + 1)],
                                    start=(j_ == 0), stop=(j_ == JB - 1),
                                    skip_group_check=True,
                                )

                    for g in range(G):
                        vals = [val_pool.tile([128, L], F32, tag="val", name="val")
                                for _ in range(2)]
                        group_vals.append(vals)
                        for j in range(JB):
                            for ih in range(2):
                                pts = []
                                los = []
                                for pr in range(2):          # head pairs (0,1),(2,3)
                                    lo = qkt_pool.tile([128, L], F32, tag="lo",
                                                       name="lo")
                                    los.append(lo)
                                    for hi in range(2):
                                        hh = 2 * pr + hi
                                        ps = slice(32 * hh, 32 * (hh + 1))
                                        js = slice(128 * j, 128 * (j + 1))
                                        is_ = slice(512 * ih, 512 * (ih + 1))
                                        terms = ((k1a[g], q1a[g]),
                                                 (k1a[g], q2a[g]),
                                                 (k2a[g], q1a[g]))
                                        for ti, (kt_, qt_) in enumerate(terms):
                                            nc.tensor.matmul(
                                                lo[:, 512 * hi:512 * (hi + 1)],
                                                kt_[ps, js], qt_[ps, is_],
                                                start=(ti == 0), stop=(ti == 2),
                                                tile_position=(32 * hh, 0),
                                                skip_group_check=True,
                                            )
                                # previous superstep's PV lands on the PE queue
                                # here, between this superstep's QKT and the
                                # next one's, so PE never stalls waiting on exp
                                if pending_pv is not None:
                                    emit_pv(*pending_pv)
                                for pr in range(2):
                                    pt = pt_pool.tile([128, L], BF16, tag="pt",
                                                      name="pt")
                                    nc.scalar.activation(pt[:], los[pr][:], EXP,
                                                         bias=logm_t[:, j:j + 1])
                                    pts.append(pt)
                                pending_pv = (vals, pts, g, j, ih)
                        if g + 1 < G:
                            # flush group g's last PV now so its drain can
                            # overlap group g+1's supersteps
                            emit_pv(*pending_pv)
                            pending_pv = None
                            _drain_group(g, vals)
                    emit_pv(*pending_pv)
                    pending_pv = None
                    _drain_group(G - 1, group_vals[G - 1])

                # ---- phase C: normalizers + scaling -----------------------------
                with tc.tile_pool(name=f"rpsum{_rep}", bufs=1, space="PSUM") as rps:
                    for g in range(G):
                        rp = rps.tile([128, L], F32, tag="rp", name="rp")
                        for ih in range(2):
                            nc.tensor.matmul(rp[:, 512 * ih:512 * (ih + 1)], sel_t[:],
                                             rm[g][:, 512 * ih:512 * (ih + 1)])
                        nc.vector.tensor_mul(valsc[g][:], valk[g][:], rp[:])

                # ---- phase D: projection + bias + mask + store ------------------
                with tc.tile_pool(name=f"projp{_rep}", bufs=4, space="PSUM") as pjp, \
                     tc.tile_pool(name=f"outp{_rep}", bufs=4) as outp:
                    for lb in range(LB):
                        ls = slice(128 * lb, 128 * (lb + 1))
                        pj = pjp.tile([128, D], F32, tag="pj", name="pj")
                        nc.tensor.matmul(pj[:], valsc[0][:, ls], wpt_t[0][:],
                                         start=True, stop=False)
                        nc.tensor.matmul(pj[:], valsc[1][:, ls], wpt_t[1][:],
                                         start=False, stop=False)
                        nc.tensor.matmul(pj[:], ones_row[:], bp_t[:],
                                         start=False, stop=True)
                        ot = outp.tile([128, D], F32, tag="ot", name="ot")
                        nc.scalar.activation(ot[:], pj[:], COPY,
                                             scale=mcols_t[:, lb:lb + 1])
                        nc.sync.dma_start(out_d[ls, :], ot[:])

    _split_multi_waits(nc)
    return nc


def _host_prep(queries, keys, values, mask, Wq, Wk, Wv, Wp, bp):
    """Shared (per-core-invariant) weight tensors + per-core input maps."""
    f32 = np.float32

    import ml_dtypes

    def bdT(W, g):
        out = np.zeros((128, 128), f32)
        for j in range(4):
            out[32 * j:32 * (j + 1), 32 * j:32 * (j + 1)] = W[4 * g + j].T
        return out

    wq = np.stack([bdT(Wq, g) for g in range(G)]).astype(f32)
    wk = np.stack([bdT(Wk, g) for g in range(G)]).astype(f32)
    wv = np.stack([bdT(Wv, g) for g in range(G)]).astype(ml_dtypes.bfloat16)
    wpt = np.ascontiguousarray(Wp.T.reshape(G, 128, D)).astype(ml_dtypes.bfloat16)
    bpr = np.asarray(bp).reshape(1, D).astype(ml_dtypes.bfloat16)
    sel = np.zeros((4, 128), ml_dtypes.bfloat16)
    for a in range(4):
        sel[a, 32 * a:32 * (a + 1)] = 1.0

    in_maps = []
    for b in range(BS):
        m = np.ascontiguousarray(mask[b, :, 0]).astype(f32)
        in_maps.append({
            "q": np.ascontiguousarray(queries[b]).astype(f32),
            "k": np.ascontiguousarray(keys[b]).astype(f32),
            "v": np.ascontiguousarray(values[b]).astype(ml_dtypes.bfloat16),
            "mcols": np.ascontiguousarray(m.reshape(JB, 128).T).astype(f32),
            "sel": sel,
            "wq": wq, "wk": wk, "wv": wv, "wpt": wpt, "bp": bpr,
        })
    return in_maps


def _run(in_maps, **kwargs):
    if "nc" not in _CACHED:
        _CACHED["nc"] = _build_nc()
    return run_bass_kernel_spmd(_CACHED["nc"], in_maps, list(range(BS)), **kwargs)


def kernel(queries, keys, values, mask, Wq, Wk, Wv, Wp, bp):
    in_maps = _host_prep(queries, keys, values, mask, Wq, Wk, Wv, Wp, bp)
    res = _run(in_maps)
    return np.stack([res.results[b]["out"] for b in range(BS)]).astype(np.float32)

